# revision 16
# baseline (speedup 1.0000x reference)
"""Group-quantized linear (fake int4 per-group dequant) GEMV on 8 Trainium2 cores.

Reference computation (all fp32):
    qw = round_half_even(clip(W, -8, 7))            # W in [-8, 7) so clip is identity
    out = (qw.reshape(O, 64, 128) * scales[:, :, None]).reshape(O, O) @ x

Sharding: column-parallel — each core owns a 1024-row slice of W/scales,
x replicated, outputs concatenated (per the tensor-parallel hint).

Device pipeline, built around the HBM stream (memory-bound problem):
  DMA   : TWO HW DGE queues (SP + Activation engines) stream the weights
          concurrently (~400 GB/s aggregate vs ~310 single-queue).  The
          per-core weight slice is shipped pre-packed (pure host-side
          layout): one 512 KiB block per group, [128 c-partitions, 1024 o],
          even groups in queue-A's linear region, odd groups in queue-B's,
          so each group lands with 4 KiB-contiguous descriptors and
          completes individually every ~1.2 us.
  DVE   : per-group quantize via the fp32 magic-number trick
          (w + 1.5*2^23) - 1.5*2^23 == round-half-even exactly, cast to
          bf16 (exact for ints in [-8, 7]); one tensor_scalar per group so
          compute trails the stream by ~1 group instead of ~1 chunk.
  PE    : per (group g, out-chunk oc) matmul acc[:, oc, g, :2] =
          qw[128c, 128o].T @ x2[128c, 2] where x2 = [x_hi | x_lo] bf16
          Dekker split of x (fp32-accurate), all accumulated in one fp32
          PSUM tile [128, 8, 64, 2].
  DVE   : epilogue out[o] = sum_{g,j} acc[o, oc, g, j] * scales[o, oc, g]:
          stage A covers groups 0..55 overlapped with the stream tail,
          stage B (groups 56..63 + combine) is 3 tiny ops after the last
          matmul.  Scales ride mid-stream, split across both queues.
  PE/DVE: transpose [128, 8] result for a contiguous output DMA
"""

import numpy as np

IN_DIM = 8192
OUT_DIM = 8192
NUM_GROUPS = 64
GROUP_SIZE = 128  # IN_DIM // NUM_GROUPS
N_CORES = 8
PER_OUT = OUT_DIM // N_CORES  # 1024
P = 128
OC_N = PER_OUT // P  # 8

MAGIC = np.float32(12582912.0)  # 1.5 * 2**23: (w + MAGIC) - MAGIC == rint(w)

GPC = 8  # groups per "chunk" (DMA issue/buffering unit)
N_CHUNKS = NUM_GROUPS // GPC
EP_SPLIT = 56  # epilogue stage-A covers groups [0, EP_SPLIT)

_cache = {}


def _split_multi_waits(nc):
    """walrus in this container accepts only ONE sync-wait per instruction;
    Tile's tail drain carries one per producer proc. Hoist extras onto
    same-engine NoOps placed immediately before — identical semantics for an
    in-order sequencer."""
    import concourse.mybir as mybir

    uid = 0
    for f in nc.m.functions:
        for blk in f.blocks:
            insts = blk.instructions
            if not any(
                i.sync_info is not None
                and i.sync_info.on_wait
                and len(i.sync_info.on_wait) > 1
                for i in insts
            ):
                continue
            new_insts = []
            for inst in insts:
                si = inst.sync_info
                if si is not None and si.on_wait and len(si.on_wait) > 1:
                    waits = list(si.on_wait)
                    for w in waits[:-1]:
                        uid += 1
                        new_insts.append(
                            mybir.InstNoOp(
                                name=f"I-waitsplit-{uid}",
                                engine=inst.engine,
                                ins=[],
                                outs=[],
                                sync_info=mybir.SyncInfo(on_wait=[w], on_update=[]),
                            )
                        )
                    inst.sync_info = mybir.SyncInfo(
                        on_wait=[waits[-1]], on_update=si.on_update
                    )
                new_insts.append(inst)
            blk.instructions = new_insts
    return nc


def build_nc(w_groups=24, q_groups=16, split_waits=True):
    import concourse.bass as bass
    import concourse.mybir as mybir
    import concourse.tile as tile
    from concourse.masks import make_identity

    f32 = mybir.dt.float32
    bf16 = mybir.dt.bfloat16
    add = mybir.AluOpType.add

    ng = NUM_GROUPS
    gelems = P * PER_OUT  # elems per group block
    half_elems = IN_DIM * PER_OUT // 2

    nc = bass.Bass()
    wa_d = nc.dram_tensor("wa", [half_elems], f32, kind="ExternalInput")
    wb_d = nc.dram_tensor("wb", [half_elems], f32, kind="ExternalInput")
    x_d = nc.dram_tensor("x", [IN_DIM], f32, kind="ExternalInput")
    sc_d = nc.dram_tensor("scales", [P, OC_N, ng], f32, kind="ExternalInput")
    out_d = nc.dram_tensor("out", [PER_OUT], f32, kind="ExternalOutput")

    with tile.TileContext(nc) as tc:
        with (
            tc.tile_pool(name="singles", bufs=1) as singles,
            tc.tile_pool(name="w", bufs=w_groups) as wpool,
            tc.tile_pool(name="q", bufs=q_groups) as qpool,
            tc.tile_pool(name="psum", bufs=1, space="PSUM") as psum,
        ):
            # ---- x load first on the SP queue (tiny), then weights flow.
            x_nat = singles.tile([ng, GROUP_SIZE], f32)
            nc.sync.dma_start(x_nat, x_d.rearrange("(g c) -> g c", c=GROUP_SIZE))

            # ---- weight stream: one DMA per group; even groups on the SP
            # queue (region A), odd on the Act queue (region B), so groups
            # complete pairwise every ~1.2 us and compute can trail tightly.
            sc_sb = singles.tile([P, OC_N, ng], f32)
            wtiles = []
            offa = offb = 0
            for g in range(ng):
                if g == (N_CHUNKS - 2) * GPC:
                    # scales ride mid-stream, half per queue: early enough
                    # for the stage-A epilogue, late enough not to delay
                    # the first-chunk pipeline fill
                    nc.sync.dma_start(sc_sb[:, : OC_N // 2, :], sc_d[:, : OC_N // 2, :])
                    nc.scalar.dma_start(
                        sc_sb[:, OC_N // 2 :, :], sc_d[:, OC_N // 2 :, :]
                    )
                wf = wpool.tile([P, PER_OUT], f32, tag="wf", name=f"wf{g}")
                src_shape = lambda t, o: t[o : o + gelems].rearrange(
                    "(c o) -> c o", c=P
                )
                if g % 2 == 0:
                    nc.sync.dma_start(wf, src_shape(wa_d, offa))
                    offa += gelems
                else:
                    nc.scalar.dma_start(wf, src_shape(wb_d, offb))
                    offb += gelems
                wtiles.append(wf)

            # ---- x prep: PE-transpose [ng,128] -> [128,ng], Dekker-split
            # into interleaved bf16 hi/lo [128, ng, 2].
            ident_g = singles.tile([ng, ng], f32)
            make_identity(nc, ident_g)
            ident_p = singles.tile([P, P], f32)
            make_identity(nc, ident_p)

            x_ps = psum.tile([P, ng], f32, tag="paux")
            nc.tensor.transpose(x_ps, x_nat, ident_g)
            xT = singles.tile([P, ng], f32)
            nc.vector.tensor_copy(out=xT, in_=x_ps)
            xhi = singles.tile([P, ng], bf16)
            nc.vector.tensor_copy(out=xhi, in_=xT)
            xhi32 = singles.tile([P, ng], f32)
            nc.vector.tensor_copy(out=xhi32, in_=xhi)
            xlo32 = singles.tile([P, ng], f32)
            nc.vector.tensor_tensor(xlo32, xT, xhi32, mybir.AluOpType.subtract)
            x2 = singles.tile([P, ng, 2], bf16)
            nc.vector.tensor_copy(out=x2[:, :, 0], in_=xhi)
            nc.vector.tensor_copy(out=x2[:, :, 1], in_=xlo32)

            # one fused PSUM accumulator [128, oc, g, hi/lo] (2 banks)
            acc = psum.tile([P, OC_N, ng, 2], f32, tag="pacc")
            accf = acc.rearrange("p oc g j -> p oc (g j)")

            # ---- main loop: per-group quantize + 8 matmuls
            for g in range(ng):
                qw = qpool.tile([P, PER_OUT], bf16, tag="qw", name=f"qw{g}")
                nc.vector.tensor_scalar(
                    out=qw,
                    in0=wtiles[g],
                    scalar1=float(MAGIC),
                    scalar2=-float(MAGIC),
                    op0=add,
                    op1=add,
                )
                for oc in range(OC_N):
                    nc.tensor.matmul(
                        acc[:, oc, g, :],
                        lhsT=qw[:, oc * P : (oc + 1) * P],
                        rhs=x2[:, g, :],
                        start=True,
                        stop=True,
                    )

            # ---- epilogue: out[o] = sum_{g,j} acc * scales (scales dup'd
            # over hi/lo).  Stage A (groups < EP_SPLIT) overlaps the stream
            # tail; stage B + combine run after the last matmul.
            sc2 = singles.tile([P, OC_N, ng, 2], f32)
            nc.vector.tensor_copy(out=sc2[:, :, :, 0], in_=sc_sb)
            nc.vector.tensor_copy(out=sc2[:, :, :, 1], in_=sc_sb)
            sc2f = sc2.rearrange("p oc g j -> p oc (g j)")
            es = EP_SPLIT * 2

            ysA = singles.tile([P, OC_N, es], f32)
            nc.vector.tensor_tensor(
                ysA, accf[:, :, :es], sc2f[:, :, :es], mybir.AluOpType.mult
            )
            outA = singles.tile([P, OC_N], f32)
            nc.vector.reduce_sum(
                out=outA.unsqueeze(2),
                in_=ysA,
                axis=mybir.AxisListType.X,
            )

            ysB = singles.tile([P, OC_N, ng * 2 - es], f32)
            nc.vector.tensor_tensor(
                ysB, accf[:, :, es:], sc2f[:, :, es:], mybir.AluOpType.mult
            )
            outB = singles.tile([P, OC_N], f32)
            nc.vector.reduce_sum(
                out=outB.unsqueeze(2),
                in_=ysB,
                axis=mybir.AxisListType.X,
            )
            out_sb = singles.tile([P, OC_N], f32)
            nc.vector.tensor_tensor(out_sb, outA, outB, add)

            # ---- transpose [128, oc] -> [oc, 128] for a contiguous store
            o_ps = psum.tile([OC_N, P], f32, tag="paux")
            nc.tensor.transpose(o_ps, out_sb, ident_p)
            outT = singles.tile([OC_N, P], f32)
            nc.vector.tensor_copy(out=outT, in_=o_ps)
            nc.sync.dma_start(out_d.rearrange("(oc p) -> oc p", p=P), outT)

    return _split_multi_waits(nc) if split_waits else nc


def make_in_maps(x, weights, scales):
    """Per-core input staging (host-side layout only)."""
    x = np.ascontiguousarray(np.asarray(x, dtype=np.float32))
    weights = np.asarray(weights, dtype=np.float32)
    scales = np.asarray(scales, dtype=np.float32)
    in_maps = []
    for c in range(N_CORES):
        sl = slice(c * PER_OUT, (c + 1) * PER_OUT)
        wtc = weights[sl].T  # [in_dim, per_out]
        # group block g = wtc[g*128:(g+1)*128, :]  ([128 c, 1024 o],
        # partition-contiguous); even groups -> region A, odd -> region B
        blocks = wtc.reshape(NUM_GROUPS, P, PER_OUT)
        wa = np.ascontiguousarray(blocks[0::2].ravel())
        wb = np.ascontiguousarray(blocks[1::2].ravel())
        scc = np.ascontiguousarray(
            scales[sl].reshape(OC_N, P, NUM_GROUPS).transpose(1, 0, 2)
        )
        in_maps.append({"wa": wa, "wb": wb, "x": x, "scales": scc})
    return in_maps


def kernel(x, weights, scales):
    from concourse import bass_utils

    if "nc" not in _cache:
        _cache["nc"] = build_nc()
    nc = _cache["nc"]

    in_maps = make_in_maps(x, weights, scales)
    res = bass_utils.run_bass_kernel_spmd(nc, in_maps, core_ids=list(range(N_CORES)))
    return np.concatenate([res.results[c]["out"] for c in range(N_CORES)]).astype(
        np.float32
    )


# revision 17
# speedup vs baseline: 1.0064x; 1.0064x over previous
"""Group-quantized linear (fake int4 per-group dequant) GEMV on 8 Trainium2 cores.

Reference computation (all fp32):
    qw = round_half_even(clip(W, -8, 7))            # W in [-8, 7) so clip is identity
    out = (qw.reshape(O, 64, 128) * scales[:, :, None]).reshape(O, O) @ x

Sharding: column-parallel — each core owns a 1024-row slice of W/scales,
x replicated, outputs concatenated (per the tensor-parallel hint).

Device pipeline, built around the HBM stream (memory-bound problem):
  DMA   : TWO HW DGE queues (SP + Activation engines) stream the weights
          concurrently (~400 GB/s aggregate vs ~310 single-queue).  The
          per-core weight slice is shipped pre-packed (pure host-side
          layout) into two linear regions, one per queue.  Bulk of the
          stream moves in 2 MiB "quarter" units (4 groups; 16 KiB
          partition-contiguous descriptors — measured fastest); the last
          8 groups move as 512 KiB per-group units so the compute tail
          trails the final bytes by ~2 us instead of ~12.
  DVE   : quantize via the fp32 magic-number trick (w + 1.5*2^23) -
          1.5*2^23 == round-half-even exactly, cast to bf16 (exact for
          ints in [-8, 7]); one tensor_scalar per unit.
  PE    : per (group g, out-chunk oc) matmul acc[:, oc, g, :2] =
          qw[128c, 128o].T @ x2[128c, 2] where x2 = [x_hi | x_lo] bf16
          Dekker split of x (fp32-accurate), all accumulated in one fp32
          PSUM tile [128, 8, 64, 2] (2 banks).
  DVE   : epilogue out[o] = sum_{g,j} acc[o, oc, g, j] * scales[o, oc, g]
          with hi/lo-duplicated scales: stage A (groups < 56) is emitted
          after the tail quantizes so DVE never blocks on PE mid-tail;
          stage B (groups 56..63) + combine are 3 tiny ops after the last
          matmul.  Scales ride mid-stream, split across both queues.
  PE/DVE: transpose [128, 8] result for a contiguous output DMA
"""

import numpy as np

IN_DIM = 8192
OUT_DIM = 8192
NUM_GROUPS = 64
GROUP_SIZE = 128  # IN_DIM // NUM_GROUPS
N_CORES = 8
PER_OUT = OUT_DIM // N_CORES  # 1024
P = 128
OC_N = PER_OUT // P  # 8

MAGIC = np.float32(12582912.0)  # 1.5 * 2**23: (w + MAGIC) - MAGIC == rint(w)

QUARTER = 4  # groups per bulk DMA unit (16 KiB/partition descriptors)
TAIL_GROUPS = 8  # final groups shipped one-per-DMA for a tight tail
N_QUARTERS = (NUM_GROUPS - TAIL_GROUPS) // QUARTER  # 14
EP_SPLIT = NUM_GROUPS - TAIL_GROUPS  # epilogue stage-A covers groups [0, 56)

_cache = {}


def _units():
    """(kind, group_start, n_groups) in stream order; queue alternates."""
    u = [("q", qi * QUARTER, QUARTER) for qi in range(N_QUARTERS)]
    u += [("g", EP_SPLIT + i, 1) for i in range(TAIL_GROUPS)]
    return u


def _split_multi_waits(nc):
    """walrus in this container accepts only ONE sync-wait per instruction;
    Tile's tail drain carries one per producer proc. Hoist extras onto
    same-engine NoOps placed immediately before — identical semantics for an
    in-order sequencer."""
    import concourse.mybir as mybir

    uid = 0
    for f in nc.m.functions:
        for blk in f.blocks:
            insts = blk.instructions
            if not any(
                i.sync_info is not None
                and i.sync_info.on_wait
                and len(i.sync_info.on_wait) > 1
                for i in insts
            ):
                continue
            new_insts = []
            for inst in insts:
                si = inst.sync_info
                if si is not None and si.on_wait and len(si.on_wait) > 1:
                    waits = list(si.on_wait)
                    for w in waits[:-1]:
                        uid += 1
                        new_insts.append(
                            mybir.InstNoOp(
                                name=f"I-waitsplit-{uid}",
                                engine=inst.engine,
                                ins=[],
                                outs=[],
                                sync_info=mybir.SyncInfo(on_wait=[w], on_update=[]),
                            )
                        )
                    inst.sync_info = mybir.SyncInfo(
                        on_wait=[waits[-1]], on_update=si.on_update
                    )
                new_insts.append(inst)
            blk.instructions = new_insts
    return nc


def build_nc(w_bufs=5, q_bufs=4, split_waits=True):
    import concourse.bass as bass
    import concourse.mybir as mybir
    import concourse.tile as tile
    from concourse.masks import make_identity

    f32 = mybir.dt.float32
    bf16 = mybir.dt.bfloat16
    add = mybir.AluOpType.add

    ng = NUM_GROUPS
    half_elems = IN_DIM * PER_OUT // 2

    nc = bass.Bass()
    wa_d = nc.dram_tensor("wa", [half_elems], f32, kind="ExternalInput")
    wb_d = nc.dram_tensor("wb", [half_elems], f32, kind="ExternalInput")
    x_d = nc.dram_tensor("x", [IN_DIM], f32, kind="ExternalInput")
    sc_d = nc.dram_tensor("scales", [P, OC_N, ng], f32, kind="ExternalInput")
    out_d = nc.dram_tensor("out", [PER_OUT], f32, kind="ExternalOutput")

    units = _units()

    with tile.TileContext(nc) as tc:
        with (
            tc.tile_pool(name="singles", bufs=1) as singles,
            tc.tile_pool(name="w", bufs=w_bufs) as wpool,
            tc.tile_pool(name="wt", bufs=TAIL_GROUPS) as wtpool,
            tc.tile_pool(name="q", bufs=q_bufs) as qpool,
            tc.tile_pool(name="qt", bufs=TAIL_GROUPS) as qtpool,
            tc.tile_pool(name="psum", bufs=1, space="PSUM") as psum,
        ):
            # ---- x load first on the SP queue (tiny), then weights flow.
            x_nat = singles.tile([ng, GROUP_SIZE], f32)
            nc.sync.dma_start(x_nat, x_d.rearrange("(g c) -> g c", c=GROUP_SIZE))

            # ---- weight stream: unit k on queue k%2 (A=SP, B=Act)
            sc_sb = singles.tile([P, OC_N, ng], f32)
            utiles = []
            offs = [0, 0]
            regions = [wa_d, wb_d]
            for k, (kind, gs, g) in enumerate(units):
                if kind == "g" and gs == EP_SPLIT:
                    # scales ride just before the tail, half per queue —
                    # early enough for sc2 prep, late enough to not delay
                    # the pipeline fill
                    nc.sync.dma_start(
                        sc_sb[:, : OC_N // 2, :], sc_d[:, : OC_N // 2, :]
                    )
                    nc.scalar.dma_start(
                        sc_sb[:, OC_N // 2 :, :], sc_d[:, OC_N // 2 :, :]
                    )
                pool = wpool if kind == "q" else wtpool
                wf = pool.tile(
                    [P, g, PER_OUT], f32, tag=f"wf_{kind}", name=f"wf{k}"
                )
                qe = k % 2
                eng = nc.sync if qe == 0 else nc.scalar
                n = P * g * PER_OUT
                eng.dma_start(
                    wf,
                    regions[qe][offs[qe] : offs[qe] + n].rearrange(
                        "(c g o) -> c g o", c=P, g=g
                    ),
                )
                offs[qe] += n
                utiles.append(wf)

            # ---- x prep: PE-transpose [ng,128] -> [128,ng], Dekker-split
            # into interleaved bf16 hi/lo [128, ng, 2].
            ident_g = singles.tile([ng, ng], f32)
            make_identity(nc, ident_g)
            ident_p = singles.tile([P, P], f32)
            make_identity(nc, ident_p)

            x_ps = psum.tile([P, ng], f32, tag="paux")
            nc.tensor.transpose(x_ps, x_nat, ident_g)
            xT = singles.tile([P, ng], f32)
            nc.vector.tensor_copy(out=xT, in_=x_ps)
            xhi = singles.tile([P, ng], bf16)
            nc.vector.tensor_copy(out=xhi, in_=xT)
            xhi32 = singles.tile([P, ng], f32)
            nc.vector.tensor_copy(out=xhi32, in_=xhi)
            xlo32 = singles.tile([P, ng], f32)
            nc.vector.tensor_tensor(xlo32, xT, xhi32, mybir.AluOpType.subtract)
            x2 = singles.tile([P, ng, 2], bf16)
            nc.vector.tensor_copy(out=x2[:, :, 0], in_=xhi)
            nc.vector.tensor_copy(out=x2[:, :, 1], in_=xlo32)

            # one fused PSUM accumulator [128, oc, g, hi/lo] (2 banks)
            acc = psum.tile([P, OC_N, ng, 2], f32, tag="pacc")
            accf = acc.rearrange("p oc g j -> p oc (g j)")

            # sc2 = scales duplicated over hi/lo, for flat epilogue APs
            sc2 = singles.tile([P, OC_N, ng, 2], f32)

            # ---- main loop: per-unit quantize + 8 matmuls per group
            for k, (kind, gs, g) in enumerate(units):
                qp = qpool if kind == "q" else qtpool
                qw = qp.tile(
                    [P, g, PER_OUT], bf16, tag=f"qw_{kind}", name=f"qw{k}"
                )
                nc.vector.tensor_scalar(
                    out=qw,
                    in0=utiles[k],
                    scalar1=float(MAGIC),
                    scalar2=-float(MAGIC),
                    op0=add,
                    op1=add,
                )
                for gp in range(g):
                    for oc in range(OC_N):
                        nc.tensor.matmul(
                            acc[:, oc, gs + gp, :],
                            lhsT=qw[:, gp, oc * P : (oc + 1) * P],
                            rhs=x2[:, gs + gp, :],
                            start=True,
                            stop=True,
                        )
                if k == N_QUARTERS - 1:
                    # sc2 prep slots in while the tail groups stream
                    nc.vector.tensor_copy(out=sc2[:, :, :, 0], in_=sc_sb)
                    nc.vector.tensor_copy(out=sc2[:, :, :, 1], in_=sc_sb)

            # ---- epilogue: out[o] = sum_{g,j} acc * sc2.  Stage A first
            # (groups < EP_SPLIT; all matmuls for those finished long ago),
            # stage B + combine after the final matmul.
            sc2f = sc2.rearrange("p oc g j -> p oc (g j)")
            es = EP_SPLIT * 2

            ysA = singles.tile([P, OC_N, es], f32)
            nc.vector.tensor_tensor(
                ysA, accf[:, :, :es], sc2f[:, :, :es], mybir.AluOpType.mult
            )
            outA = singles.tile([P, OC_N], f32)
            nc.vector.reduce_sum(
                out=outA.unsqueeze(2), in_=ysA, axis=mybir.AxisListType.X
            )

            ysB = singles.tile([P, OC_N, ng * 2 - es], f32)
            nc.vector.tensor_tensor(
                ysB, accf[:, :, es:], sc2f[:, :, es:], mybir.AluOpType.mult
            )
            outB = singles.tile([P, OC_N], f32)
            nc.vector.reduce_sum(
                out=outB.unsqueeze(2), in_=ysB, axis=mybir.AxisListType.X
            )
            out_sb = singles.tile([P, OC_N], f32)
            nc.vector.tensor_tensor(out_sb, outA, outB, add)

            # ---- transpose [128, oc] -> [oc, 128] for a contiguous store
            o_ps = psum.tile([OC_N, P], f32, tag="paux")
            nc.tensor.transpose(o_ps, out_sb, ident_p)
            outT = singles.tile([OC_N, P], f32)
            nc.vector.tensor_copy(out=outT, in_=o_ps)
            nc.sync.dma_start(out_d.rearrange("(oc p) -> oc p", p=P), outT)

    return _split_multi_waits(nc) if split_waits else nc


def make_in_maps(x, weights, scales):
    """Per-core input staging (host-side layout only)."""
    x = np.ascontiguousarray(np.asarray(x, dtype=np.float32))
    weights = np.asarray(weights, dtype=np.float32)
    scales = np.asarray(scales, dtype=np.float32)
    units = _units()
    in_maps = []
    for c in range(N_CORES):
        sl = slice(c * PER_OUT, (c + 1) * PER_OUT)
        wtc = weights[sl].T  # [in_dim, per_out]
        parts = [[], []]
        for k, (kind, gs, g) in enumerate(units):
            # [128 c, g, 1024 o]: partition-contiguous unit block
            blk = wtc[gs * P : (gs + g) * P, :].reshape(g, P, PER_OUT)
            parts[k % 2].append(blk.transpose(1, 0, 2).ravel())
        wa = np.ascontiguousarray(np.concatenate(parts[0]))
        wb = np.ascontiguousarray(np.concatenate(parts[1]))
        scc = np.ascontiguousarray(
            scales[sl].reshape(OC_N, P, NUM_GROUPS).transpose(1, 0, 2)
        )
        in_maps.append({"wa": wa, "wb": wb, "x": x, "scales": scc})
    return in_maps


def kernel(x, weights, scales):
    from concourse import bass_utils

    if "nc" not in _cache:
        _cache["nc"] = build_nc()
    nc = _cache["nc"]

    in_maps = make_in_maps(x, weights, scales)
    res = bass_utils.run_bass_kernel_spmd(nc, in_maps, core_ids=list(range(N_CORES)))
    return np.concatenate([res.results[c]["out"] for c in range(N_CORES)]).astype(
        np.float32
    )
